# revision 49
# baseline (speedup 1.0000x reference)
"""MultiHeadGAT Bass kernel for Trainium2 (8 NeuronCores, batch-parallel).

Math (per batch b, head h):
  Wh = x @ W[h]                      (N, F_OUT)
  s1_i = Wh @ a1, s2_j = Wh @ a2     (N,)
  z[i,j] = s1_i + s2_j + ab
  exps = exp(leaky_relu(z, 0.2)) * A
  attn[i,j] = exps[i,j] / (sum_i' exps[i',j] + eps)    (softmax over dim i!)
  out = attn @ Wh; concat heads

Key identity: exp(leaky(z)) = exp(0.2 z) * max(exp(0.8 z), 1), and both
exponentials are rank-1 separable over (i, j).  With
  e5r_i = exp(0.2 (s1_i + ab)),  e5c_j = exp(0.2 s2_j),  e4c_j = exp(0.8 s2_j)
the masked field in transposed layout (j on partitions) is
  ET[j,i] = AT[j,i] * e5c_j * e5r_i * max(e5r_i^4 * e4c_j, 1)
which one custom DVE op computes per 128-row strip, bf16 out, with the
column-softmax denominator d_j = sum_i ET[j,i] accumulated for free.
TensorE then computes outT[o,i] += (Wh[j,o]/(d_j+eps)) . ET[j,i].
"""

import numpy as np
import ml_dtypes
from operator import add

import concourse.bass as bass
import concourse.bacc as bacc
import concourse.mybir as mybir
import concourse.tile as tile
import concourse.dve_ops as dve_ops_mod
from concourse.dve_spec import (Spec, Src0, Src1, C0, C1, C2, One, sq, maxx,
                                lower, _has_src1)
from concourse.dve_uop import DveOpSpec
from concourse.bass_utils import run_bass_kernel_spmd

B, N, F_IN, F_OUT, H = 8, 1024, 128, 64, 4
EPS = 1e-7
NEG_SLOPE = 0.2
NCORES = 8
NSTRIP = N // 128  # 8 j-strips per core

F32 = mybir.dt.float32
BF16 = mybir.dt.bfloat16
nbf16 = ml_dtypes.bfloat16


# --------------------------------------------------------------------------
# custom DVE op: ET = Src0 * Src1 * C1 * max(Src1^4 * C0, imm2);  d += sum(ET)
#   Src0 = AT strip (mask, bf16), Src1 = e5r broadcast field (fp32)
#   C0 = e4c per-partition, C1 = e5c per-partition, imm2 = EPS accum seed
# --------------------------------------------------------------------------
def _gat_ref(in0, in1, c0, c1, c2):
    a = np.asarray(in0, np.float32)
    e5r = np.asarray(in1, np.float32)
    P = a.shape[0]
    e4c = np.broadcast_to(np.asarray(c0, np.float32).reshape(-1, 1), (P, 1))
    e5c = np.broadcast_to(np.asarray(c1, np.float32).reshape(-1, 1), (P, 1))
    m = np.maximum((e5r ** 4) * e4c, np.float32(1.0))
    body = (a * e5r * e5c * m).astype(np.float32)
    return body, np.float32(c2) + body.reshape(P, -1).sum(axis=-1, keepdims=True)


def _register_gat_op():
    name = "GAT_EXPS_MASK_REDUCE"
    for o in dve_ops_mod.OPS:
        if o.name == name:
            return o
    m = maxx(sq(sq(Src1)) * C0, One)
    spec = Spec(body=Src0 * Src1 * C1 * m, accum=add, accum_init=C2,
                reference=_gat_ref)
    shas = {}
    for ver in ("v3", "v4"):
        tmp = DveOpSpec(name=name, opcode=1, uops=lower(spec, ver=ver),
                        rd1_en=_has_src1(spec))
        shas[ver] = tmp.sha(ver)
    op = dve_ops_mod.DveOp(name, spec, False, shas)
    dve_ops_mod.OPS.append(op)
    dve_ops_mod.CUSTOM_DVE_SPECS[name] = spec
    dve_ops_mod._SUB_OPCODE_FOR_NAME[name] = (
        dve_ops_mod._CUSTOM_DVE_ROW_BASE + len(dve_ops_mod.OPS) - 1
    )
    assert dve_ops_mod._SUB_OPCODE_FOR_NAME[name] < 0x20
    return op


GAT_OP = _register_gat_op()


# --------------------------------------------------------------------------
# device program (SPMD; same program on all 8 cores, per-core data differs)
# --------------------------------------------------------------------------
def build_nc():
    nc = bacc.Bacc("TRN2", target_bir_lowering=False, debug=False,
                   enable_asserts=False, num_devices=NCORES)

    at_d = nc.dram_tensor("at", [N, N], BF16, kind="ExternalInput").ap()
    xt_d = nc.dram_tensor("xt", [F_IN, N], BF16, kind="ExternalInput").ap()
    wcat_d = nc.dram_tensor("wcat", [F_IN, H * F_OUT + 2 * H], BF16,
                            kind="ExternalInput").ap()
    w1s_d = nc.dram_tensor("w1s", [F_IN, 97], BF16, kind="ExternalInput").ap()
    bcol_d = nc.dram_tensor("bcol", [1, H], F32, kind="ExternalInput").ap()
    ot_d = nc.dram_tensor("ot", [H, F_OUT, N], F32, kind="ExternalOutput").ap()

    HF = H * F_OUT  # 256

    with tile.TileContext(nc) as tc:
        with (
            tc.tile_pool(name="const", bufs=1) as cpool,
            tc.tile_pool(name="whsb", bufs=NSTRIP) as whpool,
            tc.tile_pool(name="cols", bufs=NSTRIP) as colpool,
            tc.tile_pool(name="work", bufs=4) as wpool,
            tc.tile_pool(name="et", bufs=1) as etpool,
            tc.tile_pool(name="small", bufs=4) as spool,
            tc.tile_pool(name="ps1", bufs=1, space="PSUM") as ps1,
            tc.tile_pool(name="psw", bufs=2, space="PSUM") as psw,
            tc.tile_pool(name="psot", bufs=2, space="PSUM") as psot,
        ):
            # ---- phase 0: load small tensors -------------------------------
            xt = cpool.tile([F_IN, N], BF16, tag="xt")
            wcat = cpool.tile([F_IN, HF + 2 * H], BF16, tag="wcat")
            w1s = cpool.tile([F_IN, 97], BF16, tag="w1s")
            bcol = cpool.tile([1, H], F32, tag="bcol", padded_shape=[128, H])
            nc.sync.dma_start(xt[:, 0:512], xt_d[:, 0:512])
            nc.sync.dma_start(xt[:, 512:1024], xt_d[:, 512:1024])
            nc.sync.dma_start(wcat[:], wcat_d[:])
            nc.sync.dma_start(w1s[:], w1s_d[:])
            nc.sync.dma_start(bcol[:], bcol_d[:])

            # preload the Exp ACT table immediately (overlaps input DMAs)
            warm = cpool.tile([1, 8], F32, tag="warm",
                              padded_shape=[128, 8])
            nc.vector.memset(warm[:], 0.0)
            nc.scalar.activation(warm[0:1, :], warm[0:1, :],
                                 mybir.ActivationFunctionType.Exp)
            # dummy broadcast to absorb the Pool queue's first-op latency
            warmb = cpool.tile([128, 8], F32, tag="warmb")
            nc.gpsimd.partition_broadcast(warmb[:], warm[0:1, :])

            # ---- phase 1+2 interleaved: head-0 deps first ------------------
            e5r_bc = [None] * H
            wh_sb = [None] * NSTRIP
            e5c_sb = [None] * NSTRIP
            e4c_sb = [None] * NSTRIP

            def emit_srow(h):
                srow = ps1.tile([1, N], F32, tag="srow", name=f"srow{h}")
                nc.tensor.matmul(srow[0:1, 0:512], w1s[:, 32 * h:32 * h + 1],
                                 xt[:, 0:512], start=True, stop=True)
                nc.tensor.matmul(srow[0:1, 512:1024],
                                 w1s[:, 32 * h:32 * h + 1],
                                 xt[:, 512:1024], start=True, stop=True)
                r0 = cpool.tile([1, N], F32, tag=f"e5row0_{h}",
                                name=f"e5row0_{h}", padded_shape=[128, N])
                t = cpool.tile([128, N], F32, tag=f"e5rbc{h}",
                               name=f"e5rbc{h}")
                if h == 0:
                    # pipeline exp/broadcast halves; separate half tiles so
                    # each broadcast depends only on its own exp
                    for ci, ns in enumerate((slice(0, 512), slice(512, 1024))):
                        rh0 = cpool.tile([1, 512], F32, tag=f"e5r0h{ci}",
                                         name=f"e5r0h{ci}",
                                         padded_shape=[128, 512])
                        nc.scalar.activation(
                            rh0[0:1, :], srow[0:1, ns],
                            mybir.ActivationFunctionType.Exp,
                            bias=bcol[0:1, h:h + 1], scale=1.0)
                        nc.gpsimd.partition_broadcast(t[:, ns], rh0[0:1, :])
                else:
                    nc.scalar.activation(r0[0:1, :], srow[0:1, :],
                                         mybir.ActivationFunctionType.Exp,
                                         bias=bcol[0:1, h:h + 1], scale=1.0)
                    nc.gpsimd.partition_broadcast(t[:], r0[0:1, :])
                e5r_bc[h] = t

            def emit_whsc(js):
                whsc = psw.tile([128, HF + 2 * H], F32, tag="whsc",
                                name=f"whsc{js}")
                nc.tensor.matmul(whsc[:], xt[:, js * 128:(js + 1) * 128],
                                 wcat[:], start=True, stop=True)
                wh = whpool.tile([128, HF + 2 * H], F32, tag="wh",
                                 name=f"wh{js}")
                nc.scalar.copy(wh[:], whsc[:])
                wh_sb[js] = wh
                e5c = colpool.tile([128, H], F32, tag="e5c", name=f"e5c{js}")
                nc.scalar.activation(e5c[:], whsc[:, HF:HF + H],
                                     mybir.ActivationFunctionType.Exp)
                e5c_sb[js] = e5c
                # Square is in the exp table set -> no table reload
                e2c = spool.tile([128, H], F32, tag="e2c", name=f"e2c{js}")
                nc.scalar.activation(e2c[:], e5c[:],
                                     mybir.ActivationFunctionType.Square)
                e4c = colpool.tile([128, H], F32, tag="e4c", name=f"e4c{js}")
                nc.scalar.activation(e4c[:], e2c[:],
                                     mybir.ActivationFunctionType.Square)
                e4c_sb[js] = e4c

            emit_srow(0)
            emit_whsc(0)
            emit_whsc(1)
            for h in range(1, H):
                emit_srow(h)
            for js in range(2, NSTRIP):
                emit_whsc(js)

            # ---- phase 3: field compute (vector), head-major ---------------
            at_sb = {}
            for js in range(NSTRIP):
                at_t = etpool.tile([128, N], BF16, tag=f"at{js}",
                                   name=f"at{js}")
                nc.sync.dma_start(at_t[:], at_d[js * 128:(js + 1) * 128, :])
                at_sb[js] = at_t

            ot_ps = [psot.tile([128, N], F32, tag="ot", name=f"otps{i}")
                     for i in range(2)]
            for hi, h in enumerate([0, 2, 1, 3]):
                pair, po = h // 2, (h % 2) * 64
                tp = (0, po) if po else None
                js_seq = list(range(NSTRIP))
                for pos, js in enumerate(js_seq):
                    et = etpool.tile([128, N], BF16, tag=f"et{h}_{js}",
                                     name=f"et{h}_{js}")
                    ds = etpool.tile([128, 1], F32, tag=f"d{h}_{js}",
                                     name=f"d{h}_{js}")
                    rs = etpool.tile([128, 1], F32, tag=f"r{h}_{js}",
                                     name=f"r{h}_{js}")
                    nc.vector._custom_dve(
                        GAT_OP, out=et[:], in0=at_sb[js][:],
                        in1=e5r_bc[h][:],
                        s0=e4c_sb[js][:, h:h + 1],
                        s1=e5c_sb[js][:, h:h + 1],
                        imm2=EPS, accum_out=ds[:])
                    nc.vector.reciprocal(rs[:], ds[:])
                    whp = etpool.tile([128, F_OUT], BF16, tag=f"whp{h}_{js}",
                                      name=f"whp{h}_{js}")
                    nc.scalar.mul(whp[:],
                                  wh_sb[js][:, h * F_OUT:(h + 1) * F_OUT],
                                  rs[:])
                    for nch in range(2):
                        ns = slice(nch * 512, (nch + 1) * 512)
                        nc.tensor.matmul(
                            ot_ps[pair][po:po + 64, ns], whp[:], et[:, ns],
                            start=(pos == 0), stop=(pos == NSTRIP - 1),
                            tile_position=tp)

            # ---- phase 4: write out (chunked to overlap copy and DMA) ------
            for pair in range(2):
                ot_sb = cpool.tile([128, N], F32, tag=f"otsb{pair}",
                                   name=f"otsb{pair}")
                for nch in range(2):
                    ns = slice(nch * 512, (nch + 1) * 512)
                    # last pair: DVE is idle, split copies ACT/DVE in parallel
                    if pair == 1 and nch == 1:
                        nc.vector.tensor_copy(ot_sb[:, ns], ot_ps[pair][:, ns])
                    else:
                        nc.scalar.copy(ot_sb[:, ns], ot_ps[pair][:, ns])
                    for hh in range(2):
                        h, po = pair * 2 + hh, hh * 64
                        eng = nc.sync if hh == 0 else nc.gpsimd
                        eng.dma_start(ot_d[h][:, ns], ot_sb[po:po + 64, ns])

    nc.compile()
    return nc


# --------------------------------------------------------------------------
# host-side pre/post processing
# --------------------------------------------------------------------------
def prep_in_maps(A, x, W, a_w, a_b):
    A = np.asarray(A, np.float32)
    x = np.asarray(x, np.float32)
    W = np.asarray(W, np.float32)
    a_w = np.asarray(a_w, np.float32)
    a_b = np.asarray(a_b, np.float32)

    a1, a2 = a_w[:, :F_OUT], a_w[:, F_OUT:]
    # w1s columns spread to 0/32/64/96 so s-rows land on those partitions
    w1sc = (NEG_SLOPE * np.einsum("hfo,ho->fh", W, a1)).astype(np.float32)
    w1s = np.zeros((F_IN, 97), np.float32)
    w1s[:, 0::32] = w1sc
    w1s = w1s.astype(nbf16)
    w2s = (NEG_SLOPE * np.einsum("hfo,ho->fh", W, a2)).astype(np.float32)
    w4 = W.transpose(1, 0, 2).reshape(F_IN, H * F_OUT)  # [f, h*F_OUT+o]
    w2raw = np.einsum("hfo,ho->fh", W, a2).astype(np.float32)
    wcat = np.concatenate([w4, w2s, w2raw], axis=1).astype(nbf16)
    bcol = (NEG_SLOPE * a_b).reshape(1, H).astype(np.float32)

    at_c = [np.ascontiguousarray(A[c].T).astype(nbf16) for c in range(NCORES)]

    in_maps = []
    for c in range(NCORES):
        in_maps.append({
            "at": at_c[c],
            "xt": np.ascontiguousarray(x[c].T).astype(nbf16),
            "wcat": wcat,
            "w1s": w1s,
            "bcol": bcol,
        })
    return in_maps


def postprocess(results):
    out = np.empty((B, N, H * F_OUT), np.float32)
    for c in range(NCORES):
        ot = results[c]["ot"]  # [H, F_OUT, N]
        out[c] = ot.transpose(2, 0, 1).reshape(N, H * F_OUT)
    return out


_NC_CACHE = None


def get_nc():
    global _NC_CACHE
    if _NC_CACHE is None:
        _NC_CACHE = build_nc()
    return _NC_CACHE


def kernel(A, x, W, a_w, a_b):
    nc = get_nc()
    in_maps = prep_in_maps(A, x, W, a_w, a_b)
    res = run_bass_kernel_spmd(nc, in_maps, core_ids=list(range(NCORES)))
    return postprocess(res.results)
